# revision 1
# baseline (speedup 1.0000x reference)
"""Trainium2 Bass kernel for DeepAveragingLSTMNetwork on 8 NeuronCores.

Strategy:
  - GloVe table sharded row-wise (vocab-parallel): core m holds rows
    [m*50000, (m+1)*50000). Host computes, per core, the list of word
    positions whose index falls in its shard (padded to 768 with weight-0
    slots); the core indirect-DMA-gathers those rows and reduces them with
    a weights-vector matmul into a [1, 300] partial sum.
  - Words sharded (data-parallel) for the char-LSTM: core m runs the
    16-step LSTM for words [m*512, (m+1)*512) with hidden state laid out
    [128 hidden, 512 words]. The char-embedding lookup is fused into the
    input projection: CW = char_embed @ W_ih.T ([100, 512]) is computed on
    device once, and per-step gate pre-activations accumulate
    CW^T-slice @ one-hot(chars) + W_hh^T-slice @ h in PSUM. The one-hot
    [100 chars, 8192 (step, word)] is built on device from the index row
    via a K=1 broadcast matmul + vector is_equal against an iota column.
  - Per-core partial sums (glove [1,300], mean-pooled h [1,128]) are packed
    into a [1, 512] vector, AllGathered across the 8 cores, reduced with a
    ones*(1/4096) matmul (which also transposes into [128,4] chunks), and
    the tiny MLP runs replicated on every core. Core 0's [1, 2] output is
    returned.
"""

import os
import sys

sys.path.insert(0, "/opt/trn_rl_repo")

import numpy as np
import ml_dtypes

import concourse.bass as bass
import concourse.tile as tile
from concourse import bacc, mybir
from concourse.bass_utils import run_bass_kernel_spmd

F32 = mybir.dt.float32
F32R = mybir.dt.float32r
BF16 = mybir.dt.bfloat16
I32 = mybir.dt.int32

N_CORES = 8
GLOVE_VOCAB, GLOVE_DIM = 400000, 300
CHAR_VOCAB, CHAR_EMB, CHAR_HID = 100, 50, 128
N_WORDS, WORD_LEN = 4096, 16
HIDDEN, OUT = 512, 2

V_SHARD = GLOVE_VOCAB // N_CORES          # 50000
W_SHARD = N_WORDS // N_CORES              # 512
G_CAP = 768                               # padded gather capacity (6 x 128)
G_TILES = G_CAP // 128
HW2 = W_SHARD // 2                        # 256, LSTM half-batch width
C_PACK = 512                              # packed partial-vector width (>= 428)

# LSTM matmul precision mode: "f32" (exact, 4 cyc/col) or "bf16" (half
# storage, full-rate matmul). ("f32r" needs producer-side rounding and is
# reduced precision anyway — walrus rejects unrounded producers.)
MODE = os.environ.get("BASS_LSTM_MODE", "bf16")


def _build(mode):
    mmdt = BF16 if mode == "bf16" else F32

    nc = bacc.Bacc(
        "TRN2",
        target_bir_lowering=False,
        debug=False,
        enable_asserts=False,
        num_devices=N_CORES,
    )

    def din(name, shape, dt):
        return nc.dram_tensor(name, shape, dt, kind="ExternalInput").ap()

    # per-core inputs
    gshard = din("glove_shard", [V_SHARD, GLOVE_DIM], F32)
    gidx_in = din("g_idx", [128, G_TILES], I32)
    gw_in = din("g_w", [128, G_TILES], F32)
    ci_in = din("ci", [1, WORD_LEN * W_SHARD], mmdt)
    # replicated weights / constants
    ceT_in = din("ceT", [CHAR_EMB, CHAR_VOCAB], F32)
    wihT_in = din("wihT", [CHAR_EMB, 4 * CHAR_HID], F32)
    whhT_in = din("whhT", [CHAR_HID, 4 * CHAR_HID], mmdt)
    bbc_in = din("b_bcast", [CHAR_VOCAB, 4 * CHAR_HID], F32)
    ones100_in = din("ones100", [1, CHAR_VOCAB], mmdt)
    iota_in = din("iota128", [128, 1], F32)
    ident_in = din("identity", [128, 128], F32)
    red8_in = din("red8", [N_CORES, 1], F32)
    fc1wT_in = din("fc1wT", [128, 4 * C_PACK], F32)
    fc2wT_in = din("fc2wT", [128, 4 * OUT], F32)
    fc2b_in = din("fc2b", [1, OUT], F32)

    out_ap = nc.dram_tensor("out", [1, OUT], F32, kind="ExternalOutput").ap()

    def mm(ap):
        # matmul-operand dtype view for the LSTM path
        if mode == "f32r":
            return ap.bitcast(F32R)
        return ap

    TT = mybir.AluOpType
    AF = mybir.ActivationFunctionType

    with tile.TileContext(nc) as tc:
        with (
            tc.tile_pool(name="const", bufs=1) as cp,
            tc.tile_pool(name="work", bufs=3) as wp,
            tc.tile_pool(name="ohp", bufs=WORD_LEN) as ohp,
            tc.tile_pool(name="psA", bufs=2, space="PSUM") as ppA,
            tc.tile_pool(name="ps", bufs=3, space="PSUM") as pp,
            tc.tile_pool(name="dram", bufs=1, space="DRAM") as dp,
        ):
            # ---- load constants into SBUF (round-robin DMA queues) ----
            _qs = [nc.sync, nc.scalar, nc.gpsimd]
            _qi = [0]

            def load(name, ap_in, shape, dt):
                t = cp.tile(shape, dt, tag=name)
                _qs[_qi[0] % len(_qs)].dma_start(out=t[:], in_=ap_in[:])
                _qi[0] += 1
                return t

            gidx = load("gidx", gidx_in, [128, G_TILES], I32)
            gw = load("gw", gw_in, [128, G_TILES], F32)
            ci = load("ci", ci_in, [1, WORD_LEN * W_SHARD], mmdt)
            ceT = load("ceT", ceT_in, [CHAR_EMB, CHAR_VOCAB], F32)
            wihT = load("wihT", wihT_in, [CHAR_EMB, 4 * CHAR_HID], F32)
            whhT = load("whhT", whhT_in, [CHAR_HID, 4 * CHAR_HID], mmdt)
            bbc = load("bbc", bbc_in, [CHAR_VOCAB, 4 * CHAR_HID], F32)
            ones100 = load("ones100", ones100_in, [1, CHAR_VOCAB], mmdt)
            iota = load("iota", iota_in, [128, 1], F32)
            ident = load("ident", ident_in, [128, 128], F32)
            red8 = load("red8", red8_in, [N_CORES, 1], F32)
            fc1wT = load("fc1wT", fc1wT_in, [128, 4 * C_PACK], F32)
            fc2wT = load("fc2wT", fc2wT_in, [128, 4 * OUT], F32)
            fc2b = load("fc2b", fc2b_in, [1, OUT], F32)

            # ---- partial-sum vector (written as pieces become ready) ----
            par = cp.tile([1, C_PACK], F32, tag="par")
            nc.vector.memset(par[:], 0.0)
            # avg[428] must come out as 1.0 after the AllGather-sum * (1/4096):
            # every core contributes 512. fc1wT row 428 carries fc1_b, so the
            # fc1 matmul includes its bias and relu needs no bias operand.
            nc.vector.memset(par[:, 428:429], float(W_SHARD))

            # ---- CW = char_embed @ W_ih.T + b  -> [100, 512] ----
            # (one-hot columns sum to 1, so adding b to every row of CW folds
            # the gate bias into the x-projection — activations need no bias)
            ps_cw = ppA.tile([CHAR_VOCAB, 4 * CHAR_HID], F32, tag="psA")
            nc.tensor.matmul(ps_cw[:], lhsT=ceT[:], rhs=wihT[:], start=True, stop=True)
            cw = cp.tile([CHAR_VOCAB, 4 * CHAR_HID], mmdt, tag="cw")
            nc.vector.tensor_add(out=cw[:], in0=ps_cw[:], in1=bbc[:])

            # ---- one-hot per step: OH_t[c, w] = (ci[t*512+w] == c) ----
            # (separate tiles so LSTM step t only depends on its own slice)
            oh = []
            for t in range(WORD_LEN):
                ps_bc = ppA.tile([CHAR_VOCAB, W_SHARD], F32, tag="psA")
                nc.tensor.matmul(
                    ps_bc[:],
                    lhsT=mm(ones100[:]),
                    rhs=mm(ci[:, t * W_SHARD : (t + 1) * W_SHARD]),
                    start=True,
                    stop=True,
                )
                oh_t = ohp.tile([CHAR_VOCAB, W_SHARD], mmdt, tag="oh")
                nc.vector.tensor_tensor(
                    out=oh_t[:],
                    in0=ps_bc[:],
                    in1=iota[0:CHAR_VOCAB, 0:1].to_broadcast([CHAR_VOCAB, W_SHARD]),
                    op=TT.is_equal,
                )
                oh.append(oh_t)

            # ---- glove gather + weighted reduce ----
            ps_gl = ppA.tile([1, GLOVE_DIM], F32, tag="psA")
            for j in range(G_TILES):
                gt = wp.tile([128, GLOVE_DIM], F32, tag="gl")
                nc.gpsimd.indirect_dma_start(
                    out=gt[:],
                    out_offset=None,
                    in_=gshard[:],
                    in_offset=bass.IndirectOffsetOnAxis(ap=gidx[:, j : j + 1], axis=0),
                )
                nc.tensor.matmul(
                    ps_gl[:],
                    lhsT=gw[:, j : j + 1],
                    rhs=gt[:],
                    start=(j == 0),
                    stop=(j == G_TILES - 1),
                )
            nc.vector.tensor_copy(out=par[:, 0:GLOVE_DIM], in_=ps_gl[:])

            # Dummy tiny collective issued early: it overlaps the LSTM and
            # warms the CC/ncfw path so the real AllGather at the tail is
            # cheaper. Its output lands in par's zero-padded columns, which
            # are annihilated by fc1's zero-padded rows.
            if os.environ.get("BASS_WARM_CC", "1") == "1":
                wz = cp.tile([1, 8], F32, tag="wz")
                nc.vector.memset(wz[:], 0.0)
                warm_in = dp.tile([1, 8], F32, tag="warm_in")
                warm_out = dp.tile([N_CORES, 8], F32, tag="warm_out")
                nc.sync.dma_start(out=warm_in[:], in_=wz[:])
                nc.gpsimd.collective_compute(
                    "AllGather",
                    TT.bypass,
                    replica_groups=[list(range(N_CORES))],
                    ins=[warm_in.opt()],
                    outs=[warm_out.opt()],
                )
                nc.sync.dma_start(out=par[:, C_PACK - 8 : C_PACK], in_=warm_out[0:1, :])

            # ---- LSTM over 16 steps, two independent half-batches ----
            # Gate pre-activations for one half-step live in one [128, 1024]
            # PSUM tile (2 banks) laid out [i | f | o | g] so one ACTIVATE
            # covers the three sigmoids and one covers the tanh.
            cdt = BF16 if os.environ.get("BASS_C_BF16", "0") == "1" else F32
            PS_OFF = [0, 1, 3, 2]  # weight-gate order i,f,g,o -> psum quarter
            h_prev = [None, None]
            c_prev = [None, None]
            for t in range(WORD_LEN):
                for hb in range(2):
                    gp = pp.tile([128, 4 * HW2], F32, tag="ps")
                    for g in range(4):
                        q = PS_OFF[g]
                        dst = gp[:, q * HW2 : (q + 1) * HW2]
                        nc.tensor.matmul(
                            dst,
                            lhsT=mm(cw[:, g * 128 : (g + 1) * 128]),
                            rhs=mm(oh[t][:, hb * HW2 : (hb + 1) * HW2]),
                            start=True,
                            stop=(t == 0),
                        )
                        if t > 0:
                            nc.tensor.matmul(
                                dst,
                                lhsT=mm(whhT[:, g * 128 : (g + 1) * 128]),
                                rhs=mm(h_prev[hb][:]),
                                start=False,
                                stop=True,
                            )
                    sig = wp.tile([128, 3 * HW2], mmdt, tag=f"sig_{hb}")
                    nc.scalar.activation(sig[:], gp[:, 0 : 3 * HW2], AF.Sigmoid)
                    a_i = sig[:, 0:HW2]
                    a_f = sig[:, HW2 : 2 * HW2]
                    a_o = sig[:, 2 * HW2 : 3 * HW2]
                    a_g = wp.tile([128, HW2], mmdt, tag=f"tg_{hb}")
                    nc.scalar.activation(a_g[:], gp[:, 3 * HW2 : 4 * HW2], AF.Tanh)
                    c_new = wp.tile([128, HW2], cdt, tag=f"c_{hb}")
                    if t == 0:
                        nc.vector.tensor_tensor(
                            out=c_new[:], in0=a_i[:], in1=a_g[:], op=TT.mult
                        )
                    else:
                        t1 = wp.tile([128, HW2], cdt, tag=f"t1_{hb}")
                        nc.vector.tensor_tensor(
                            out=t1[:], in0=a_f[:], in1=c_prev[hb][:], op=TT.mult
                        )
                        t2 = wp.tile([128, HW2], cdt, tag=f"t2_{hb}")
                        nc.vector.tensor_tensor(
                            out=t2[:], in0=a_i[:], in1=a_g[:], op=TT.mult
                        )
                        nc.vector.tensor_add(out=c_new[:], in0=t1[:], in1=t2[:])
                    th = wp.tile([128, HW2], mmdt, tag=f"th_{hb}")
                    nc.scalar.activation(th[:], c_new[:], AF.Tanh)
                    h_new = wp.tile([128, HW2], mmdt, tag=f"h_{hb}")
                    nc.vector.tensor_tensor(
                        out=h_new[:], in0=a_o[:], in1=th[:], op=TT.mult
                    )
                    h_prev[hb] = h_new
                    c_prev[hb] = c_new

            # ---- mean-pool h over words: [128, 1], then transpose to [1, 128] ----
            hs = []
            for hb in range(2):
                r = wp.tile([128, 1], F32, tag=f"hs_{hb}")
                nc.vector.tensor_reduce(
                    out=r[:], in_=h_prev[hb][:], axis=mybir.AxisListType.X, op=TT.add
                )
                hs.append(r)
            hsum = wp.tile([128, 1], F32, tag="hsum")
            nc.vector.tensor_add(out=hsum[:], in0=hs[0][:], in1=hs[1][:])
            ps_ht = ppA.tile([1, 128], F32, tag="psA")
            nc.tensor.matmul(ps_ht[:], lhsT=hsum[:], rhs=ident[:], start=True, stop=True)
            nc.vector.tensor_copy(
                out=par[:, GLOVE_DIM : GLOVE_DIM + CHAR_HID], in_=ps_ht[:]
            )

            # ---- AllGather partials, reduce+transpose into avgT [128, 4] ----
            par_d = dp.tile([1, C_PACK], F32, tag="par_d")
            ag_d = dp.tile([N_CORES, C_PACK], F32, tag="ag_d")
            nc.sync.dma_start(out=par_d[:], in_=par[:])
            nc.gpsimd.collective_compute(
                "AllGather",
                TT.bypass,
                replica_groups=[list(range(N_CORES))],
                ins=[par_d.opt()],
                outs=[ag_d.opt()],
            )
            ag = cp.tile([N_CORES, C_PACK], F32, tag="ag")
            nc.sync.dma_start(out=ag[:], in_=ag_d[:])

            ps_avg = ppA.tile([128, 4], F32, tag="psA")
            for kc in range(4):
                nc.tensor.matmul(
                    ps_avg[:, kc : kc + 1],
                    lhsT=ag[:, kc * 128 : (kc + 1) * 128],
                    rhs=red8[:],
                    start=True,
                    stop=True,
                )
            avgT = cp.tile([128, 4], F32, tag="avgT")
            nc.vector.tensor_copy(out=avgT[:], in_=ps_avg[:])

            # ---- MLP: hid = relu(fc1_w @ avg + fc1_b); out = fc2_w @ hid + fc2_b ----
            ps_hid = ppA.tile([128, 4], F32, tag="psA")
            for mc in range(4):
                for kc in range(4):
                    nc.tensor.matmul(
                        ps_hid[:, mc : mc + 1],
                        lhsT=fc1wT[:, kc * C_PACK + mc * 128 : kc * C_PACK + (mc + 1) * 128],
                        rhs=avgT[:, kc : kc + 1],
                        start=(kc == 0),
                        stop=(kc == 3),
                    )
            hid = cp.tile([128, 4], F32, tag="hid")
            nc.vector.tensor_scalar_max(out=hid[:], in0=ps_hid[:], scalar1=0.0)
            ps_out = ppA.tile([1, OUT], F32, tag="psA")
            for kc in range(4):
                nc.tensor.matmul(
                    ps_out[:],
                    lhsT=hid[:, kc : kc + 1],
                    rhs=fc2wT[:, kc * OUT : (kc + 1) * OUT],
                    start=(kc == 0),
                    stop=(kc == 3),
                )
            res = cp.tile([1, OUT], F32, tag="res")
            nc.vector.tensor_add(out=res[:], in0=ps_out[:], in1=fc2b[:])
            nc.sync.dma_start(out=out_ap[:], in_=res[:])

    nc.compile()
    return nc


_NC_CACHE = {}


def _get_nc(mode):
    key = (mode, os.environ.get("BASS_C_BF16", "0"), os.environ.get("BASS_WARM_CC", "1"))
    if key not in _NC_CACHE:
        _NC_CACHE[key] = _build(mode)
    return _NC_CACHE[key]


def make_in_maps(
    word_indices,
    char_indices,
    glove_table,
    char_embed,
    W_ih,
    W_hh,
    b_ih,
    b_hh,
    fc1_w,
    fc1_b,
    fc2_w,
    fc2_b,
    mode=MODE,
):
    mmnp = ml_dtypes.bfloat16 if mode == "bf16" else np.float32

    wi = np.asarray(word_indices).astype(np.int64).reshape(N_WORDS)
    ci = np.asarray(char_indices).astype(np.int64).reshape(N_WORDS, WORD_LEN)
    glove_table = np.asarray(glove_table, dtype=np.float32)
    char_embed = np.asarray(char_embed, dtype=np.float32)
    W_ih = np.asarray(W_ih, dtype=np.float32)
    W_hh = np.asarray(W_hh, dtype=np.float32)
    b = (np.asarray(b_ih, dtype=np.float32) + np.asarray(b_hh, dtype=np.float32))
    fc1_w = np.asarray(fc1_w, dtype=np.float32)
    fc1_b = np.asarray(fc1_b, dtype=np.float32)
    fc2_w = np.asarray(fc2_w, dtype=np.float32)
    fc2_b = np.asarray(fc2_b, dtype=np.float32)

    # replicated tensors
    ceT = np.ascontiguousarray(char_embed.T)                       # [50, 100]
    wihT = np.ascontiguousarray(W_ih.T)                            # [50, 512]
    whhT = np.ascontiguousarray(W_hh.T).astype(mmnp)               # [128, 512]
    b_bcast = np.broadcast_to(b, (CHAR_VOCAB, 4 * CHAR_HID)).copy()
    ones100 = np.ones((1, CHAR_VOCAB), dtype=mmnp)
    iota128 = np.arange(128, dtype=np.float32).reshape(128, 1)
    identity = np.eye(128, dtype=np.float32)
    red8 = np.full((N_CORES, 1), 1.0 / N_WORDS, dtype=np.float32)
    fc1wT_pad = np.zeros((128, 4 * C_PACK), dtype=np.float32)
    fc1T_ext = np.zeros((4 * 128, HIDDEN), dtype=np.float32)
    fc1T_ext[: GLOVE_DIM + CHAR_HID] = fc1_w.T                     # rows 0..427
    fc1T_ext[GLOVE_DIM + CHAR_HID] = fc1_b                         # bias row 428
    for kc in range(4):
        fc1wT_pad[:, kc * C_PACK : kc * C_PACK + HIDDEN] = fc1T_ext[kc * 128 : (kc + 1) * 128]
    fc2T = fc2_w.T                                                 # [512, 2]
    fc2wT = np.zeros((128, 4 * OUT), dtype=np.float32)
    for kc in range(4):
        fc2wT[:, kc * OUT : (kc + 1) * OUT] = fc2T[kc * 128 : (kc + 1) * 128]
    fc2b = fc2_b.reshape(1, OUT)

    rep = dict(
        ceT=ceT, wihT=wihT, whhT=whhT, b_bcast=b_bcast, ones100=ones100,
        iota128=iota128, identity=identity, red8=red8,
        fc1wT=fc1wT_pad, fc2wT=fc2wT, fc2b=fc2b,
    )

    in_maps = []
    for m in range(N_CORES):
        shard = np.ascontiguousarray(glove_table[m * V_SHARD : (m + 1) * V_SHARD])
        sel = np.nonzero((wi >= m * V_SHARD) & (wi < (m + 1) * V_SHARD))[0]
        loc = (wi[sel] - m * V_SHARD).astype(np.int32)
        n = loc.shape[0]
        assert n <= G_CAP, f"core {m}: {n} rows exceed capacity {G_CAP}"
        g_idx = np.zeros(G_CAP, dtype=np.int32)
        g_idx[:n] = np.sort(loc)
        g_w = np.zeros(G_CAP, dtype=np.float32)
        g_w[:n] = 1.0
        # column-major packing: tile j holds slots [j*128, (j+1)*128)
        g_idx = np.ascontiguousarray(g_idx.reshape(G_TILES, 128).T)
        g_w = np.ascontiguousarray(g_w.reshape(G_TILES, 128).T)
        ci_m = ci[m * W_SHARD : (m + 1) * W_SHARD]                 # [512, 16]
        ci_t = np.ascontiguousarray(ci_m.T).reshape(1, WORD_LEN * W_SHARD)
        in_maps.append(
            dict(
                glove_shard=shard,
                g_idx=g_idx,
                g_w=g_w,
                ci=ci_t.astype(mmnp),
                **rep,
            )
        )
    return in_maps


def run(in_maps, mode=MODE, **kw):
    nc = _get_nc(mode)
    return nc, run_bass_kernel_spmd(nc, in_maps, list(range(N_CORES)), **kw)


def kernel(**inputs):
    in_maps = make_in_maps(**inputs)
    _, res = run(in_maps)
    return np.asarray(res.results[0]["out"])

